# revision 1
# baseline (speedup 1.0000x reference)
"""Trainium2 Bass kernel for a 3-layer GCN encoder over two graphs (x, y).

Dense-adjacency formulation:
  GCNConv(h) = D^-1/2 (A+I) D^-1/2 (h @ W) + b
  With Acnt the self-loop-augmented adjacency-count matrix and dinv = deg^-1/2:
      Hhat_1   = dinv * x                   (host, shipped bf16)
      P_l      = Acnt @ Hhat_l              (PE matmul, dominant cost)
      S_l      = dinv * P_l                 (dst-side norm)
      z_l      = S_l @ W_l + b_l            (PE matmul; bias via rank-1 matmul)
      Hhat_l+1 = dinv * relu(z_l)           (src-side norm of next layer)
  Output layer: out = S_3 @ W_3 + b_3.

Sharding: all 8 cores form one replica group; each core owns a 1280-row
(1250 real) dst shard of BOTH graphs. Acnt^T is streamed from HBM as bf16
(exact small-integer counts); Hhat for both graphs is SBUF-resident and
replicated with a single 8-rank AllGather per hidden layer.

Node ids are renumbered into a padded space of 10240 = 8*1280 so all tiles
are 128-multiples and the AllGather output is directly the packed SBUF
image of Hhat.
"""

import numpy as np
import ml_dtypes

import concourse.bass as bass
import concourse.tile as tile
from concourse import bacc, mybir
import concourse.bass_utils as bass_utils
from concourse.masks import make_identity

BF16 = ml_dtypes.bfloat16

P = 128          # partitions / tile edge
NC = 8           # cores
N_NODES = 10000
SHARD = 1250     # real nodes per core (per graph)
SHP = 1280       # padded nodes per core
NPAD = NC * SHP  # 10240
KT = NPAD // P   # 80 k-tiles over src nodes
MT = SHP // P    # 10 m-tiles per graph per core
F = 256          # in/hidden feature width
FO = 128         # output feature width

_NC_CACHE = {}


# ----------------------------------------------------------------------------
# Host-side graph preprocessing (index/static work only)
# ----------------------------------------------------------------------------

def _pad_ids(n):
    return (n // SHARD) * SHP + (n % SHARD)


def _prep_graph(x, edge_index, Ws, bs):
    """Returns (per-core list of 8 slab tensors, h1_img, w_imgs, b_rows, dinv_pad)."""
    src = edge_index[0].astype(np.int64)
    dst = edge_index[1].astype(np.int64)
    loop = np.arange(N_NODES, dtype=np.int64)
    src = np.concatenate([src, loop])
    dst = np.concatenate([dst, loop])
    sp = _pad_ids(src)
    dp = _pad_ids(dst)

    deg = np.zeros(NPAD, np.float32)
    np.add.at(deg, dp, np.float32(1.0))
    dinv = np.zeros(NPAD, np.float32)
    nz = deg > 0
    dinv[nz] = 1.0 / np.sqrt(deg[nz])

    at = np.zeros((NPAD, NPAD), np.float32)   # [src, dst] = A^T counts
    np.add.at(at, (sp, dp), np.float32(1.0))

    h1 = np.zeros((NPAD, F), np.float32)
    h1[_pad_ids(loop)] = x * dinv[_pad_ids(loop)][:, None]
    h1_img = np.ascontiguousarray(
        h1.reshape(KT, P, F).transpose(1, 0, 2).reshape(P, KT * F)
    ).astype(BF16)

    def w_img(W, fo):
        kf = W.shape[0] // P
        return np.ascontiguousarray(
            W.reshape(kf, P, fo).transpose(1, 0, 2).reshape(P, kf * fo)
        ).astype(BF16)

    slabs = []
    for g in range(NC):
        shard = at[:, g * SHP:(g + 1) * SHP]  # [NPAD src, SHP dst]
        slab = np.ascontiguousarray(
            shard.reshape(KT, P, MT, P).transpose(2, 1, 0, 3).reshape(MT, P, KT * P)
        ).astype(BF16)
        slabs.append(slab)
    w_imgs = [w_img(Ws[0], F), w_img(Ws[1], F), w_img(Ws[2], FO)]
    b_rows = [bs[0].reshape(1, F).astype(BF16),
              bs[1].reshape(1, F).astype(BF16),
              bs[2].reshape(1, FO).astype(BF16)]
    return slabs, h1_img, w_imgs, b_rows, dinv


def prep_in_maps(x, x_edge_index, y, y_edge_index,
                 W1x, b1x, W2x, b2x, W3x, b3x,
                 W1y, b1y, W2y, b2y, W3y, b3y):
    sx, h1x, wx, bx, dx = _prep_graph(
        np.asarray(x, np.float32), np.asarray(x_edge_index),
        (np.asarray(W1x), np.asarray(W2x), np.asarray(W3x)),
        (np.asarray(b1x), np.asarray(b2x), np.asarray(b3x)))
    sy, h1y, wy, by, dy = _prep_graph(
        np.asarray(y, np.float32), np.asarray(y_edge_index),
        (np.asarray(W1y), np.asarray(W2y), np.asarray(W3y)),
        (np.asarray(b1y), np.asarray(b2y), np.asarray(b3y)))
    maps = []
    for c in range(NC):
        dvx = dx[c * SHP:(c + 1) * SHP].reshape(MT, P).T   # [P, MT]
        dvy = dy[c * SHP:(c + 1) * SHP].reshape(MT, P).T
        maps.append({
            "at": np.stack([sx[c], sy[c]]),   # [2, MT, P, KT*P]
            "hx": h1x, "hy": h1y,
            "w0": wx[0], "w1": wx[1], "w2": wx[2],
            "w3": wy[0], "w4": wy[1], "w5": wy[2],
            "b0": bx[0], "b1": bx[1], "b2": bx[2],
            "b3": by[0], "b4": by[1], "b5": by[2],
            "dinv": np.ascontiguousarray(
                np.concatenate([dvx, dvy], axis=1)).astype(np.float32),  # [P, 2*MT]
        })
    return maps


def _unshard(z_imgs, graph):
    """8 per-core [P, 2*MT*FO] images -> [N_NODES, FO] for graph 0(x)/1(y)."""
    rows = []
    for z in z_imgs:
        zi = z.reshape(P, 2 * MT, FO)[:, graph * MT:(graph + 1) * MT, :]
        r = zi.transpose(1, 0, 2).reshape(SHP, FO)
        rows.append(r[:SHARD])
    return np.concatenate(rows, axis=0)


# ----------------------------------------------------------------------------
# Device kernel
# ----------------------------------------------------------------------------

def _build_nc():
    if "nc" in _NC_CACHE:
        return _NC_CACHE["nc"]
    nc = bacc.Bacc("TRN2", target_bir_lowering=False, debug=False, num_devices=NC)
    dt = mybir.dt

    at = nc.dram_tensor("at", [2, MT, P, KT * P], dt.bfloat16, kind="ExternalInput").ap()
    hx = nc.dram_tensor("hx", [P, KT * F], dt.bfloat16, kind="ExternalInput").ap()
    hy = nc.dram_tensor("hy", [P, KT * F], dt.bfloat16, kind="ExternalInput").ap()
    w_ap = [nc.dram_tensor(f"w{i}", [P, 2 * (FO if i % 3 == 2 else F)], dt.bfloat16,
                           kind="ExternalInput").ap() for i in range(6)]
    b_ap = [nc.dram_tensor(f"b{i}", [1, FO if i % 3 == 2 else F], dt.bfloat16,
                           kind="ExternalInput").ap() for i in range(6)]
    dinv = nc.dram_tensor("dinv", [P, 2 * MT], dt.float32, kind="ExternalInput").ap()
    zout = nc.dram_tensor("z", [P, 2 * MT * FO], dt.float32, kind="ExternalOutput").ap()

    groups = [list(range(NC))]

    with tile.TileContext(nc) as tc:
        with (
            tc.tile_pool(name="persist", bufs=1) as pers,
            tc.tile_pool(name="aslab", bufs=3) as apool,
            tc.tile_pool(name="work", bufs=4) as wk,
            tc.tile_pool(name="pagg", bufs=2, space="PSUM") as pagg,
            tc.tile_pool(name="ptr", bufs=2, space="PSUM") as ptr,
            tc.tile_pool(name="pg", bufs=2, space="PSUM") as pg,
            tc.tile_pool(name="dram", bufs=1, space="DRAM") as dp,
        ):
            Hg = [pers.tile([P, KT * F], dt.bfloat16, name="Hx"),
                  pers.tile([P, KT * F], dt.bfloat16, name="Hy")]
            Hown = pers.tile([P, 2 * MT * F], dt.bfloat16)
            Zsb = pers.tile([P, 2 * MT * FO], dt.float32)
            Wt = [pers.tile([P, 2 * (FO if i % 3 == 2 else F)], dt.bfloat16,
                            name=f"wt{i}") for i in range(6)]
            Bt = [pers.tile([1, FO if i % 3 == 2 else F], dt.bfloat16, name=f"bt{i}")
                  for i in range(6)]
            Dv = pers.tile([P, 2 * MT], dt.float32)
            ident = pers.tile([P, P], dt.bfloat16)
            ones = pers.tile([1, P], dt.bfloat16)

            make_identity(nc, ident[:])
            nc.gpsimd.memset(ones[:], 1.0)
            # chunked initial H loads so layer-1 matmuls can start early
            CH = KT * F // 4
            for r in range(4):
                nc.sync.dma_start(Hg[0][:, r * CH:(r + 1) * CH],
                                  hx[:, r * CH:(r + 1) * CH])
            for r in range(4):
                nc.sync.dma_start(Hg[1][:, r * CH:(r + 1) * CH],
                                  hy[:, r * CH:(r + 1) * CH])
            for i in range(6):
                nc.sync.dma_start(Wt[i][:], w_ap[i])
                nc.sync.dma_start(Bt[i][:], b_ap[i])
            nc.sync.dma_start(Dv[:], dinv)

            for layer in range(3):
                fo = FO if layer == 2 else F
                for g in range(2):
                    H = Hg[g]
                    Wl = Wt[3 * g + layer]
                    Bl = Bt[3 * g + layer]
                    for m in range(MT):
                        gm = g * MT + m
                        a_slab = apool.tile([P, KT * P], dt.bfloat16, tag="aslab")
                        # scalar-engine HWDGE queue: keeps A-slab streaming off
                        # the sync queue that carries H/W/B and AG reloads
                        nc.scalar.dma_start(a_slab[:], at[g, m])
                        pP = pagg.tile([P, F], dt.float32, tag="agg")
                        for k in range(KT):
                            nc.tensor.matmul(
                                pP[:],
                                lhsT=a_slab[:, k * P:(k + 1) * P],
                                rhs=H[:, k * F:(k + 1) * F],
                                start=(k == 0),
                                stop=(k == KT - 1),
                            )
                        S = wk.tile([P, F], dt.bfloat16, tag="S")
                        nc.vector.tensor_scalar_mul(S[:], pP[:], Dv[:, gm:gm + 1])
                        gps = pg.tile([P, fo], dt.float32, tag="g")
                        for kf in range(2):
                            pT = ptr.tile([P, P], dt.bfloat16, tag="tr")
                            nc.tensor.transpose(
                                pT[:], S[:, kf * P:(kf + 1) * P], ident[:]
                            )
                            STk = wk.tile([P, P], dt.bfloat16, tag="ST")
                            nc.vector.tensor_copy(STk[:], pT[:])
                            nc.tensor.matmul(
                                gps[:],
                                lhsT=STk[:],
                                rhs=Wl[:, kf * fo:(kf + 1) * fo],
                                start=(kf == 0),
                                stop=False,
                            )
                        nc.tensor.matmul(
                            gps[:],
                            lhsT=ones[:1, :],
                            rhs=Bl[:1, :fo],
                            start=False,
                            stop=True,
                        )
                        if layer < 2:
                            nc.scalar.activation(
                                Hown[:, gm * F:(gm + 1) * F],
                                gps[:],
                                mybir.ActivationFunctionType.Relu,
                                scale=Dv[:, gm:gm + 1],
                            )
                        else:
                            nc.vector.tensor_copy(
                                Zsb[:, gm * FO:(gm + 1) * FO], gps[:]
                            )
                    if layer < 2:
                        # Two half-AllGathers per graph, fired after m=4 and
                        # m=9 (emitted here, after the full m-loop, but each
                        # depends only on its 5 Hown tiles so Tile lets the
                        # first half fly mid-loop). Each half's latency hides
                        # under the remaining compute of this graph and the
                        # other graph's m-loop.
                        W2F = MT * F        # 2560 cols per rank in H
                        HW2 = W2F // 2      # 1280 cols per half
                        for half in range(2):
                            agin = dp.tile([P, HW2], dt.bfloat16,
                                           tag=f"agin{layer}{g}{half}")
                            agout = dp.tile([NC * P, HW2], dt.bfloat16,
                                            tag=f"agout{layer}{g}{half}")
                            nc.sync.dma_start(
                                agin[:],
                                Hown[:, g * W2F + half * HW2:
                                     g * W2F + (half + 1) * HW2])
                            nc.gpsimd.collective_compute(
                                "AllGather",
                                mybir.AluOpType.bypass,
                                replica_groups=groups,
                                ins=[agin[:].opt()],
                                outs=[agout[:].opt()],
                            )
                            for r in range(NC):
                                # gpsimd queue: reloads must not delay the
                                # next collective's input DMA on the sync ring
                                nc.gpsimd.dma_start(
                                    Hg[g][:, r * W2F + half * HW2:
                                          r * W2F + (half + 1) * HW2],
                                    agout[r * P:(r + 1) * P, :],
                                )
            nc.sync.dma_start(zout, Zsb[:])
    nc.compile()
    _NC_CACHE["nc"] = nc
    return nc


# ----------------------------------------------------------------------------
# Entry point
# ----------------------------------------------------------------------------

def kernel(x, x_edge_index, y, y_edge_index,
           W1x, b1x, W2x, b2x, W3x, b3x,
           W1y, b1y, W2y, b2y, W3y, b3y,
           _trace=False, _trace_cores=None):
    in_maps = prep_in_maps(x, x_edge_index, y, y_edge_index,
                           W1x, b1x, W2x, b2x, W3x, b3x,
                           W1y, b1y, W2y, b2y, W3y, b3y)
    nc = _build_nc()
    kw = {}
    if _trace:
        kw = dict(trace=True, trace_cores=_trace_cores or [0])
    res = bass_utils.run_bass_kernel_spmd(
        nc, in_maps, core_ids=list(range(NC)), **kw
    )
    z = [res.results[c]["z"] for c in range(NC)]
    out_x = _unshard(z, 0)
    out_y = _unshard(z, 1)
    if _trace:
        kernel._last_result = res
    return out_x, out_y



# revision 8
# speedup vs baseline: 1.3101x; 1.3101x over previous
"""Trainium2 Bass kernel for a 3-layer GCN encoder over two graphs (x, y).

Dense-adjacency formulation:
  GCNConv(h) = D^-1/2 (A+I) D^-1/2 (h @ W) + b
  With Acnt the self-loop-augmented adjacency-count matrix and dinv = deg^-1/2:
      Hhat_1   = dinv * x                   (host, shipped bf16)
      P_l      = Acnt @ Hhat_l              (PE matmul, dominant cost)
      S_l      = dinv * P_l                 (dst-side norm)
      z_l      = S_l @ W_l + b_l            (PE matmul; bias via rank-1 matmul)
      Hhat_l+1 = dinv * relu(z_l)           (src-side norm of next layer)
  Output layer: out = S_3 @ W_3 + b_3.

Sharding: all 8 cores form one replica group; each core owns a 1280-row
(1250 real) dst shard of BOTH graphs. Acnt^T is streamed from HBM as bf16
(exact small-integer counts); Hhat for both graphs is SBUF-resident and
replicated with a single 8-rank AllGather per hidden layer.

Node ids are renumbered into a padded space of 10240 = 8*1280 so all tiles
are 128-multiples and the AllGather output is directly the packed SBUF
image of Hhat.
"""

import numpy as np
import ml_dtypes

import concourse.bass as bass
import concourse.tile as tile
from concourse import bacc, mybir
import concourse.bass_utils as bass_utils
from concourse.masks import make_identity

BF16 = ml_dtypes.bfloat16
FP8 = ml_dtypes.float8_e4m3  # adjacency counts are <= 2: exact in e4m3

P = 128          # partitions / tile edge
NC = 8           # cores
N_NODES = 10000
SHARD = 1250     # real nodes per core (per graph)
SHP = 1280       # padded nodes per core
NPAD = NC * SHP  # 10240
KT = NPAD // P   # 80 k-tiles over src nodes
MT = SHP // P    # 10 m-tiles per graph per core
F = 256          # in/hidden feature width
FO = 128         # output feature width

_NC_CACHE = {}


# ----------------------------------------------------------------------------
# Host-side graph preprocessing (index/static work only)
# ----------------------------------------------------------------------------

def _pad_ids(n):
    return (n // SHARD) * SHP + (n % SHARD)


def _prep_graph(x, edge_index, Ws, bs):
    """Returns (per-core list of 8 slab tensors, h1_img, w_imgs, b_rows, dinv_pad)."""
    src = edge_index[0].astype(np.int64)
    dst = edge_index[1].astype(np.int64)
    loop = np.arange(N_NODES, dtype=np.int64)
    src = np.concatenate([src, loop])
    dst = np.concatenate([dst, loop])
    sp = _pad_ids(src)
    dp = _pad_ids(dst)

    deg = np.zeros(NPAD, np.float32)
    np.add.at(deg, dp, np.float32(1.0))
    dinv = np.zeros(NPAD, np.float32)
    nz = deg > 0
    dinv[nz] = 1.0 / np.sqrt(deg[nz])

    at = np.zeros((NPAD, NPAD), np.float32)   # [src, dst] = A^T counts
    np.add.at(at, (sp, dp), np.float32(1.0))

    h1 = np.zeros((NPAD, F), np.float32)
    h1[_pad_ids(loop)] = x * dinv[_pad_ids(loop)][:, None]
    h1_img = np.ascontiguousarray(
        h1.reshape(KT, P, F).transpose(1, 0, 2).reshape(P, KT * F)
    ).astype(BF16)

    def w_img(W, fo):
        kf = W.shape[0] // P
        return np.ascontiguousarray(
            W.reshape(kf, P, fo).transpose(1, 0, 2).reshape(P, kf * fo)
        ).astype(BF16)

    slabs = []
    for g in range(NC):
        shard = at[:, g * SHP:(g + 1) * SHP]  # [NPAD src, SHP dst]
        slab = np.ascontiguousarray(
            shard.reshape(KT, P, MT, P).transpose(2, 1, 0, 3).reshape(MT, P, KT * P)
        ).astype(FP8)
        slabs.append(slab)
    w_imgs = [w_img(Ws[0], F), w_img(Ws[1], F), w_img(Ws[2], FO)]
    b_rows = [bs[0].reshape(1, F).astype(BF16),
              bs[1].reshape(1, F).astype(BF16),
              bs[2].reshape(1, FO).astype(BF16)]
    return slabs, h1_img, w_imgs, b_rows, dinv


def prep_in_maps(x, x_edge_index, y, y_edge_index,
                 W1x, b1x, W2x, b2x, W3x, b3x,
                 W1y, b1y, W2y, b2y, W3y, b3y):
    sx, h1x, wx, bx, dx = _prep_graph(
        np.asarray(x, np.float32), np.asarray(x_edge_index),
        (np.asarray(W1x), np.asarray(W2x), np.asarray(W3x)),
        (np.asarray(b1x), np.asarray(b2x), np.asarray(b3x)))
    sy, h1y, wy, by, dy = _prep_graph(
        np.asarray(y, np.float32), np.asarray(y_edge_index),
        (np.asarray(W1y), np.asarray(W2y), np.asarray(W3y)),
        (np.asarray(b1y), np.asarray(b2y), np.asarray(b3y)))
    maps = []
    for c in range(NC):
        dvx = dx[c * SHP:(c + 1) * SHP].reshape(MT, P).T   # [P, MT]
        dvy = dy[c * SHP:(c + 1) * SHP].reshape(MT, P).T
        maps.append({
            "at": np.stack([sx[c], sy[c]]),   # [2, MT, P, KT*P]
            "hx": h1x, "hy": h1y,
            "w0": wx[0], "w1": wx[1], "w2": wx[2],
            "w3": wy[0], "w4": wy[1], "w5": wy[2],
            "b0": bx[0], "b1": bx[1], "b2": bx[2],
            "b3": by[0], "b4": by[1], "b5": by[2],
            "dinv": np.ascontiguousarray(
                np.concatenate([dvx, dvy], axis=1)).astype(np.float32),  # [P, 2*MT]
        })
    return maps


def _unshard(z_imgs, graph):
    """8 per-core [P, 2*MT*FO] images -> [N_NODES, FO] for graph 0(x)/1(y)."""
    rows = []
    for z in z_imgs:
        zi = z.reshape(P, 2 * MT, FO)[:, graph * MT:(graph + 1) * MT, :]
        r = zi.transpose(1, 0, 2).reshape(SHP, FO)
        rows.append(r[:SHARD])
    return np.concatenate(rows, axis=0)


# ----------------------------------------------------------------------------
# Device kernel
# ----------------------------------------------------------------------------

def _build_nc():
    if "nc" in _NC_CACHE:
        return _NC_CACHE["nc"]
    nc = bacc.Bacc("TRN2", target_bir_lowering=False, debug=False, num_devices=NC)
    dt = mybir.dt

    at = nc.dram_tensor("at", [2, MT, P, KT * P], dt.float8e4, kind="ExternalInput").ap()
    hx = nc.dram_tensor("hx", [P, KT * F], dt.bfloat16, kind="ExternalInput").ap()
    hy = nc.dram_tensor("hy", [P, KT * F], dt.bfloat16, kind="ExternalInput").ap()
    w_ap = [nc.dram_tensor(f"w{i}", [P, 2 * (FO if i % 3 == 2 else F)], dt.bfloat16,
                           kind="ExternalInput").ap() for i in range(6)]
    b_ap = [nc.dram_tensor(f"b{i}", [1, FO if i % 3 == 2 else F], dt.bfloat16,
                           kind="ExternalInput").ap() for i in range(6)]
    dinv = nc.dram_tensor("dinv", [P, 2 * MT], dt.float32, kind="ExternalInput").ap()
    zout = nc.dram_tensor("z", [P, 2 * MT * FO], dt.float32, kind="ExternalOutput").ap()

    groups = [list(range(NC))]

    with tile.TileContext(nc) as tc:
        with (
            tc.tile_pool(name="persist", bufs=1) as pers,
            tc.tile_pool(name="aslab", bufs=6) as apool,
            tc.tile_pool(name="work", bufs=4) as wk,
            tc.tile_pool(name="pagg", bufs=2, space="PSUM") as pagg,
            tc.tile_pool(name="ptr", bufs=2, space="PSUM") as ptr,
            tc.tile_pool(name="pg", bufs=2, space="PSUM") as pg,
            tc.tile_pool(name="dram", bufs=1, space="DRAM") as dp,
        ):
            Hg = [pers.tile([P, KT * F], dt.bfloat16, name="Hx"),
                  pers.tile([P, KT * F], dt.bfloat16, name="Hy")]
            Hown = pers.tile([P, 2 * MT * F], dt.bfloat16)
            Wt = [pers.tile([P, 2 * (FO if i % 3 == 2 else F)], dt.bfloat16,
                            name=f"wt{i}") for i in range(6)]
            Bt = [pers.tile([1, FO if i % 3 == 2 else F], dt.bfloat16, name=f"bt{i}")
                  for i in range(6)]
            Dv = pers.tile([P, 2 * MT], dt.float32)
            ident = pers.tile([P, P], dt.bfloat16)
            ones = pers.tile([1, P], dt.bfloat16)

            make_identity(nc, ident[:])
            nc.gpsimd.memset(ones[:], 1.0)
            # small operands first on the scalar queue so they're not behind H
            for i in range(6):
                nc.scalar.dma_start(Wt[i][:], w_ap[i])
                nc.scalar.dma_start(Bt[i][:], b_ap[i])
            nc.scalar.dma_start(Dv[:], dinv)
            # chunked initial H loads so layer-1 matmuls can start early
            CH = KT * F // 4
            for r in range(4):
                nc.sync.dma_start(Hg[0][:, r * CH:(r + 1) * CH],
                                  hx[:, r * CH:(r + 1) * CH])
            for r in range(4):
                nc.sync.dma_start(Hg[1][:, r * CH:(r + 1) * CH],
                                  hy[:, r * CH:(r + 1) * CH])

            # A-slab prefetch, software-pipelined 6 deep on the scalar queue:
            # slab i+6's doorbell is emitted at iteration i, so boundary
            # stalls in the scalar stream can't delay the slab the PE needs
            # next. Pool WAR deps (bufs=6) pace the stream automatically.
            slab_tiles = []

            def prefetch_slab():
                i = len(slab_tiles)
                if i >= 3 * 2 * MT:
                    return
                g, m = (i // MT) % 2, i % MT
                t = apool.tile([P, KT * P], dt.float8e4, tag="aslab",
                               name=f"aslab{i}")
                nc.scalar.dma_start(t[:], at[g, m])
                slab_tiles.append(t)

            for _ in range(6):
                prefetch_slab()

            for layer in range(3):
                fo = FO if layer == 2 else F
                for g in range(2):
                    H = Hg[g]
                    Wl = Wt[3 * g + layer]
                    Bl = Bt[3 * g + layer]
                    for m in range(MT):
                        gm = g * MT + m
                        a_slab = slab_tiles[(layer * 2 + g) * MT + m]
                        pP = pagg.tile([P, F], dt.float32, tag="agg")
                        for k in range(KT):
                            nc.tensor.matmul(
                                pP[:],
                                lhsT=a_slab[:, k * P:(k + 1) * P],
                                rhs=H[:, k * F:(k + 1) * F],
                                start=(k == 0),
                                stop=(k == KT - 1),
                            )
                        prefetch_slab()
                        S = wk.tile([P, F], dt.bfloat16, tag="S")
                        nc.vector.tensor_scalar_mul(S[:], pP[:], Dv[:, gm:gm + 1])
                        gps = pg.tile([P, fo], dt.float32, tag="g")
                        for kf in range(2):
                            pT = ptr.tile([P, P], dt.bfloat16, tag="tr")
                            nc.tensor.transpose(
                                pT[:], S[:, kf * P:(kf + 1) * P], ident[:]
                            )
                            STk = wk.tile([P, P], dt.bfloat16, tag="ST")
                            nc.vector.tensor_copy(STk[:], pT[:])
                            nc.tensor.matmul(
                                gps[:],
                                lhsT=STk[:],
                                rhs=Wl[:, kf * fo:(kf + 1) * fo],
                                start=(kf == 0),
                                stop=False,
                            )
                        nc.tensor.matmul(
                            gps[:],
                            lhsT=ones[:1, :],
                            rhs=Bl[:1, :fo],
                            start=False,
                            stop=True,
                        )
                        if layer < 2:
                            nc.scalar.activation(
                                Hown[:, gm * F:(gm + 1) * F],
                                gps[:],
                                mybir.ActivationFunctionType.Relu,
                                scale=Dv[:, gm:gm + 1],
                            )
                        else:
                            zt = wk.tile([P, FO], dt.float32, tag="zt")
                            nc.vector.tensor_copy(zt[:], gps[:])
                            nc.sync.dma_start(
                                zout[:, gm * FO:(gm + 1) * FO], zt[:]
                            )
                    if layer < 2:
                        # Two half-AllGathers per graph, fired after m=4 and
                        # m=9 (emitted here, after the full m-loop, but each
                        # depends only on its 5 Hown tiles so Tile lets the
                        # first half fly mid-loop). Each half's latency hides
                        # under the remaining compute of this graph and the
                        # other graph's m-loop.
                        W2F = MT * F        # 2560 cols per rank in H
                        HW2 = W2F // 2      # 1280 cols per half
                        for half in range(2):
                            agin = dp.tile([P, HW2], dt.bfloat16,
                                           tag=f"agin{layer}{g}{half}")
                            agout = dp.tile([NC * P, HW2], dt.bfloat16,
                                            tag=f"agout{layer}{g}{half}",
                                            addr_space="Shared")
                            nc.sync.dma_start(
                                agin[:],
                                Hown[:, g * W2F + half * HW2:
                                     g * W2F + (half + 1) * HW2])
                            nc.gpsimd.collective_compute(
                                "AllGather",
                                mybir.AluOpType.bypass,
                                replica_groups=groups,
                                ins=[agin[:].opt()],
                                outs=[agout[:].opt()],
                            )
                            for r in range(NC):
                                # gpsimd queue: reloads must not delay the
                                # next collective's input DMA on the sync ring
                                nc.gpsimd.dma_start(
                                    Hg[g][:, r * W2F + half * HW2:
                                          r * W2F + (half + 1) * HW2],
                                    agout[r * P:(r + 1) * P, :],
                                )
    nc.compile()
    _NC_CACHE["nc"] = nc
    return nc


# ----------------------------------------------------------------------------
# Entry point
# ----------------------------------------------------------------------------

def kernel(x, x_edge_index, y, y_edge_index,
           W1x, b1x, W2x, b2x, W3x, b3x,
           W1y, b1y, W2y, b2y, W3y, b3y,
           _trace=False, _trace_cores=None):
    in_maps = prep_in_maps(x, x_edge_index, y, y_edge_index,
                           W1x, b1x, W2x, b2x, W3x, b3x,
                           W1y, b1y, W2y, b2y, W3y, b3y)
    nc = _build_nc()
    kw = {}
    if _trace:
        kw = dict(trace=True, trace_cores=_trace_cores or [0])
    res = bass_utils.run_bass_kernel_spmd(
        nc, in_maps, core_ids=list(range(NC)), **kw
    )
    z = [res.results[c]["z"] for c in range(NC)]
    out_x = _unshard(z, 0)
    out_y = _unshard(z, 1)
    if _trace:
        kernel._last_result = res
    return out_x, out_y



# revision 9
# speedup vs baseline: 1.3138x; 1.0029x over previous
"""Trainium2 Bass kernel for a 3-layer GCN encoder over two graphs (x, y).

Dense-adjacency formulation:
  GCNConv(h) = D^-1/2 (A+I) D^-1/2 (h @ W) + b
  With Acnt the self-loop-augmented adjacency-count matrix and dinv = deg^-1/2:
      Hhat_1   = dinv * x                   (host, shipped bf16)
      P_l      = Acnt @ Hhat_l              (PE matmul, dominant cost)
      S_l      = dinv * P_l                 (dst-side norm)
      z_l      = S_l @ W_l + b_l            (PE matmul; bias via rank-1 matmul)
      Hhat_l+1 = dinv * relu(z_l)           (src-side norm of next layer)
  Output layer: out = S_3 @ W_3 + b_3.

Sharding: all 8 cores form one replica group; each core owns a 1280-row
(1250 real) dst shard of BOTH graphs. Acnt^T is streamed from HBM as bf16
(exact small-integer counts); Hhat for both graphs is SBUF-resident and
replicated with a single 8-rank AllGather per hidden layer.

Node ids are renumbered into a padded space of 10240 = 8*1280 so all tiles
are 128-multiples and the AllGather output is directly the packed SBUF
image of Hhat.
"""

import numpy as np
import ml_dtypes

import concourse.bass as bass
import concourse.tile as tile
from concourse import bacc, mybir
import concourse.bass_utils as bass_utils
from concourse.masks import make_identity

BF16 = ml_dtypes.bfloat16
FP8 = ml_dtypes.float8_e4m3  # adjacency counts are <= 2: exact in e4m3

P = 128          # partitions / tile edge
NC = 8           # cores
N_NODES = 10000
SHARD = 1250     # real nodes per core (per graph)
SHP = 1280       # padded nodes per core
NPAD = NC * SHP  # 10240
KT = NPAD // P   # 80 k-tiles over src nodes
MT = SHP // P    # 10 m-tiles per graph per core
F = 256          # in/hidden feature width
FO = 128         # output feature width

_NC_CACHE = {}


# ----------------------------------------------------------------------------
# Host-side graph preprocessing (index/static work only)
# ----------------------------------------------------------------------------

def _pad_ids(n):
    return (n // SHARD) * SHP + (n % SHARD)


def _prep_graph(x, edge_index, Ws, bs):
    """Returns (per-core list of 8 slab tensors, h1_img, w_imgs, b_rows, dinv_pad)."""
    src = edge_index[0].astype(np.int64)
    dst = edge_index[1].astype(np.int64)
    loop = np.arange(N_NODES, dtype=np.int64)
    src = np.concatenate([src, loop])
    dst = np.concatenate([dst, loop])
    sp = _pad_ids(src)
    dp = _pad_ids(dst)

    deg = np.zeros(NPAD, np.float32)
    np.add.at(deg, dp, np.float32(1.0))
    dinv = np.zeros(NPAD, np.float32)
    nz = deg > 0
    dinv[nz] = 1.0 / np.sqrt(deg[nz])

    at = np.zeros((NPAD, NPAD), np.float32)   # [src, dst] = A^T counts
    np.add.at(at, (sp, dp), np.float32(1.0))

    h1 = np.zeros((NPAD, F), np.float32)
    h1[_pad_ids(loop)] = x * dinv[_pad_ids(loop)][:, None]
    h1_img = np.ascontiguousarray(
        h1.reshape(KT, P, F).transpose(1, 0, 2).reshape(P, KT * F)
    ).astype(BF16)

    def w_img(W, fo):
        kf = W.shape[0] // P
        return np.ascontiguousarray(
            W.reshape(kf, P, fo).transpose(1, 0, 2).reshape(P, kf * fo)
        ).astype(BF16)

    slabs = []
    for g in range(NC):
        shard = at[:, g * SHP:(g + 1) * SHP]  # [NPAD src, SHP dst]
        slab = np.ascontiguousarray(
            shard.reshape(KT, P, MT, P).transpose(2, 1, 0, 3).reshape(MT, P, KT * P)
        ).astype(FP8)
        slabs.append(slab)
    w_imgs = [w_img(Ws[0], F), w_img(Ws[1], F), w_img(Ws[2], FO)]
    b_rows = [bs[0].reshape(1, F).astype(BF16),
              bs[1].reshape(1, F).astype(BF16),
              bs[2].reshape(1, FO).astype(BF16)]
    return slabs, h1_img, w_imgs, b_rows, dinv


def prep_in_maps(x, x_edge_index, y, y_edge_index,
                 W1x, b1x, W2x, b2x, W3x, b3x,
                 W1y, b1y, W2y, b2y, W3y, b3y):
    sx, h1x, wx, bx, dx = _prep_graph(
        np.asarray(x, np.float32), np.asarray(x_edge_index),
        (np.asarray(W1x), np.asarray(W2x), np.asarray(W3x)),
        (np.asarray(b1x), np.asarray(b2x), np.asarray(b3x)))
    sy, h1y, wy, by, dy = _prep_graph(
        np.asarray(y, np.float32), np.asarray(y_edge_index),
        (np.asarray(W1y), np.asarray(W2y), np.asarray(W3y)),
        (np.asarray(b1y), np.asarray(b2y), np.asarray(b3y)))
    maps = []
    for c in range(NC):
        dvx = dx[c * SHP:(c + 1) * SHP].reshape(MT, P).T   # [P, MT]
        dvy = dy[c * SHP:(c + 1) * SHP].reshape(MT, P).T
        maps.append({
            "at": np.stack([sx[c], sy[c]]),   # [2, MT, P, KT*P]
            "hx": h1x, "hy": h1y,
            "w0": wx[0], "w1": wx[1], "w2": wx[2],
            "w3": wy[0], "w4": wy[1], "w5": wy[2],
            "b0": bx[0], "b1": bx[1], "b2": bx[2],
            "b3": by[0], "b4": by[1], "b5": by[2],
            "dinv": np.ascontiguousarray(
                np.concatenate([dvx, dvy], axis=1)).astype(np.float32),  # [P, 2*MT]
        })
    return maps


def _unshard(z_imgs, graph):
    """8 per-core [P, 2*MT*FO] images -> [N_NODES, FO] for graph 0(x)/1(y)."""
    rows = []
    for z in z_imgs:
        zi = z.reshape(P, 2 * MT, FO)[:, graph * MT:(graph + 1) * MT, :]
        r = zi.transpose(1, 0, 2).reshape(SHP, FO)
        rows.append(r[:SHARD])
    return np.concatenate(rows, axis=0)


# ----------------------------------------------------------------------------
# Device kernel
# ----------------------------------------------------------------------------

def _build_nc():
    if "nc" in _NC_CACHE:
        return _NC_CACHE["nc"]
    nc = bacc.Bacc("TRN2", target_bir_lowering=False, debug=False, num_devices=NC)
    dt = mybir.dt

    at = nc.dram_tensor("at", [2, MT, P, KT * P], dt.float8e4, kind="ExternalInput").ap()
    hx = nc.dram_tensor("hx", [P, KT * F], dt.bfloat16, kind="ExternalInput").ap()
    hy = nc.dram_tensor("hy", [P, KT * F], dt.bfloat16, kind="ExternalInput").ap()
    w_ap = [nc.dram_tensor(f"w{i}", [P, 2 * (FO if i % 3 == 2 else F)], dt.bfloat16,
                           kind="ExternalInput").ap() for i in range(6)]
    b_ap = [nc.dram_tensor(f"b{i}", [1, FO if i % 3 == 2 else F], dt.bfloat16,
                           kind="ExternalInput").ap() for i in range(6)]
    dinv = nc.dram_tensor("dinv", [P, 2 * MT], dt.float32, kind="ExternalInput").ap()
    zout = nc.dram_tensor("z", [P, 2 * MT * FO], dt.float32, kind="ExternalOutput").ap()

    groups = [list(range(NC))]

    with tile.TileContext(nc) as tc:
        with (
            tc.tile_pool(name="persist", bufs=1) as pers,
            tc.tile_pool(name="aslab", bufs=6) as apool,
            tc.tile_pool(name="work", bufs=4) as wk,
            tc.tile_pool(name="pagg", bufs=2, space="PSUM") as pagg,
            tc.tile_pool(name="ptr", bufs=2, space="PSUM") as ptr,
            tc.tile_pool(name="pg", bufs=2, space="PSUM") as pg,
            tc.tile_pool(name="dram", bufs=1, space="DRAM") as dp,
        ):
            Hg = [pers.tile([P, KT * F], dt.bfloat16, name="Hx"),
                  pers.tile([P, KT * F], dt.bfloat16, name="Hy")]
            Hown = pers.tile([P, 2 * MT * F], dt.bfloat16)
            Wt = [pers.tile([P, 2 * (FO if i % 3 == 2 else F)], dt.bfloat16,
                            name=f"wt{i}") for i in range(6)]
            Bt = [pers.tile([1, FO if i % 3 == 2 else F], dt.bfloat16, name=f"bt{i}")
                  for i in range(6)]
            Dv = pers.tile([P, 2 * MT], dt.float32)
            ident = pers.tile([P, P], dt.bfloat16)
            ones = pers.tile([1, P], dt.bfloat16)

            make_identity(nc, ident[:])
            nc.gpsimd.memset(ones[:], 1.0)
            # A-slab prefetch, software-pipelined 6 deep on the scalar queue:
            # slab i+6's doorbell is emitted at iteration i, so boundary
            # stalls in the scalar stream can't delay the slab the PE needs
            # next. Pool WAR deps (bufs=6) pace the stream automatically.
            slab_tiles = []

            def prefetch_slab(queue=None):
                i = len(slab_tiles)
                if i >= 3 * 2 * MT:
                    return
                g, m = (i // MT) % 2, i % MT
                t = apool.tile([P, KT * P], dt.float8e4, tag="aslab",
                               name=f"aslab{i}")
                (queue or nc.scalar).dma_start(t[:], at[g, m])
                slab_tiles.append(t)

            # Startup: the PE needs Hx + slab0 + W1x/b1x/dinv within ~10us.
            # Split Hx across the sync+scalar queues, first slabs on the
            # otherwise-idle gpsimd queue, Hy (needed ~95us in) last.
            CH = KT * F // 4
            for _ in range(3):
                prefetch_slab(nc.gpsimd)
            for r in range(2):
                nc.sync.dma_start(Hg[0][:, r * CH:(r + 1) * CH],
                                  hx[:, r * CH:(r + 1) * CH])
            for r in range(2, 4):
                nc.scalar.dma_start(Hg[0][:, r * CH:(r + 1) * CH],
                                    hx[:, r * CH:(r + 1) * CH])
            nc.scalar.dma_start(Dv[:], dinv)
            for i in range(6):
                nc.scalar.dma_start(Wt[i][:], w_ap[i])
                nc.scalar.dma_start(Bt[i][:], b_ap[i])
            for r in range(4):
                nc.sync.dma_start(Hg[1][:, r * CH:(r + 1) * CH],
                                  hy[:, r * CH:(r + 1) * CH])
            for _ in range(3):
                prefetch_slab()

            for layer in range(3):
                fo = FO if layer == 2 else F
                for g in range(2):
                    H = Hg[g]
                    Wl = Wt[3 * g + layer]
                    Bl = Bt[3 * g + layer]
                    for m in range(MT):
                        gm = g * MT + m
                        a_slab = slab_tiles[(layer * 2 + g) * MT + m]
                        pP = pagg.tile([P, F], dt.float32, tag="agg")
                        for k in range(KT):
                            nc.tensor.matmul(
                                pP[:],
                                lhsT=a_slab[:, k * P:(k + 1) * P],
                                rhs=H[:, k * F:(k + 1) * F],
                                start=(k == 0),
                                stop=(k == KT - 1),
                            )
                        prefetch_slab()
                        S = wk.tile([P, F], dt.bfloat16, tag="S")
                        nc.vector.tensor_scalar_mul(S[:], pP[:], Dv[:, gm:gm + 1])
                        gps = pg.tile([P, fo], dt.float32, tag="g")
                        for kf in range(2):
                            pT = ptr.tile([P, P], dt.bfloat16, tag="tr")
                            nc.tensor.transpose(
                                pT[:], S[:, kf * P:(kf + 1) * P], ident[:]
                            )
                            STk = wk.tile([P, P], dt.bfloat16, tag="ST")
                            nc.vector.tensor_copy(STk[:], pT[:])
                            nc.tensor.matmul(
                                gps[:],
                                lhsT=STk[:],
                                rhs=Wl[:, kf * fo:(kf + 1) * fo],
                                start=(kf == 0),
                                stop=False,
                            )
                        nc.tensor.matmul(
                            gps[:],
                            lhsT=ones[:1, :],
                            rhs=Bl[:1, :fo],
                            start=False,
                            stop=True,
                        )
                        if layer < 2:
                            nc.scalar.activation(
                                Hown[:, gm * F:(gm + 1) * F],
                                gps[:],
                                mybir.ActivationFunctionType.Relu,
                                scale=Dv[:, gm:gm + 1],
                            )
                        else:
                            zt = wk.tile([P, FO], dt.float32, tag="zt")
                            nc.vector.tensor_copy(zt[:], gps[:])
                            nc.sync.dma_start(
                                zout[:, gm * FO:(gm + 1) * FO], zt[:]
                            )
                    if layer < 2:
                        # Two half-AllGathers per graph, fired after m=4 and
                        # m=9 (emitted here, after the full m-loop, but each
                        # depends only on its 5 Hown tiles so Tile lets the
                        # first half fly mid-loop). Each half's latency hides
                        # under the remaining compute of this graph and the
                        # other graph's m-loop.
                        W2F = MT * F        # 2560 cols per rank in H
                        HW2 = W2F // 2      # 1280 cols per half
                        for half in range(2):
                            agin = dp.tile([P, HW2], dt.bfloat16,
                                           tag=f"agin{layer}{g}{half}")
                            agout = dp.tile([NC * P, HW2], dt.bfloat16,
                                            tag=f"agout{layer}{g}{half}",
                                            addr_space="Shared")
                            nc.sync.dma_start(
                                agin[:],
                                Hown[:, g * W2F + half * HW2:
                                     g * W2F + (half + 1) * HW2])
                            nc.gpsimd.collective_compute(
                                "AllGather",
                                mybir.AluOpType.bypass,
                                replica_groups=groups,
                                ins=[agin[:].opt()],
                                outs=[agout[:].opt()],
                            )
                            for r in range(NC):
                                # gpsimd queue: reloads must not delay the
                                # next collective's input DMA on the sync ring
                                nc.gpsimd.dma_start(
                                    Hg[g][:, r * W2F + half * HW2:
                                          r * W2F + (half + 1) * HW2],
                                    agout[r * P:(r + 1) * P, :],
                                )
    nc.compile()
    _NC_CACHE["nc"] = nc
    return nc


# ----------------------------------------------------------------------------
# Entry point
# ----------------------------------------------------------------------------

def kernel(x, x_edge_index, y, y_edge_index,
           W1x, b1x, W2x, b2x, W3x, b3x,
           W1y, b1y, W2y, b2y, W3y, b3y,
           _trace=False, _trace_cores=None):
    in_maps = prep_in_maps(x, x_edge_index, y, y_edge_index,
                           W1x, b1x, W2x, b2x, W3x, b3x,
                           W1y, b1y, W2y, b2y, W3y, b3y)
    nc = _build_nc()
    kw = {}
    if _trace:
        kw = dict(trace=True, trace_cores=_trace_cores or [0])
    res = bass_utils.run_bass_kernel_spmd(
        nc, in_maps, core_ids=list(range(NC)), **kw
    )
    z = [res.results[c]["z"] for c in range(NC)]
    out_x = _unshard(z, 0)
    out_y = _unshard(z, 1)
    if _trace:
        kernel._last_result = res
    return out_x, out_y

